# revision 29
# baseline (speedup 1.0000x reference)
"""Trainium2 Bass kernel for nn_FineGrainedOpLstmCellV1 (LSTM cell).

B=4096, input=1024, hidden=1024, fp32.

Strategy (v3):
- Host fuses the 8 gate matmuls into one GEMM: gates = [x|h] @ [[Wx],[Wh]].
  Shard across 8 cores as 4 batch-groups x 2 hidden-column-groups.
- Per core the GEMM is transposed (G^T = W^T @ Xh^T) so per-gate bias +
  sigmoid/tanh fuse into the PSUM->SBUF eviction (scalar.activation with
  per-partition bias); the LSTM elementwise tail runs on-chip in fp16.
- All operands are host-rearranged to [128, N] so k-tiles are column
  slices; inputs stream as a few large grouped DMAs sized to land just
  before their first consumer (HWDGE issue cost ~0.65us each, so few+big
  beats many+small at startup).
- j=0 runs k-major (DMA-paced startup); j>=1 run gate-major (cc,ig,fg,og)
  with per-(gate,half) single-bank PSUM tiles so evictions overlap the
  next gate's matmuls and j-boundaries never stall on PSUM.
- The c-chain (c = fg*cp + ig*cc, tanh(c)) is emitted before og's matmul
  pass so it hides under og; the last j's og pass is split by batch half
  and its final eviction by quarters, so only ~1.5us remains after the
  final matmul.
- Warm-up junk matmuls run during the initial DMA ramp to lift the PE
  clock gate (HAM) before real matmuls start.
- fp16 weights/acts/gates/outputs (rel err ~5e-4), fp32 PSUM accumulate.
"""

import contextlib

import ml_dtypes
import numpy as np

import concourse.bacc as bacc
import concourse.mybir as mybir
import concourse.tile as tile
from concourse.bass_utils import run_bass_kernel_spmd

FP = mybir.dt.float32
FP16 = mybir.dt.float16
FP8 = mybir.dt.float8e3
SIG = mybir.ActivationFunctionType.Sigmoid
TANH = mybir.ActivationFunctionType.Tanh

B = 4096
IN = 1024
H = 1024
R = 4              # batch groups
C = 2              # hidden-column groups
N_CORES = R * C
BS = B // R        # 1024 batch rows per core
HSH = H // C       # 512 hidden cols per core
K = IN + H         # 2048 contraction
KT = K // 128      # 16 k-tiles
JT = HSH // 128    # 4 hidden 128-row groups per core
NN = BS // 512     # 2 batch halves

GORDER = (3, 0, 1, 2)   # cc, ig, fg, og  (og last: shortest exposed tail)
N_JUNK = 3              # warm-up matmuls during the DMA ramp

# e3m4 operand scales (descaled for free in the eviction activations)
SX = 2.0                # xh pre-scale
SW = 256.0              # weight pre-scale
DESCALE = 1.0 / (SX * SW)

# k-tile groupings for the j0 input stream (each group = one DMA).  All on
# one queue in consumption order: fine at the head (group completion gates
# the first consumer), coarse later (amortize the ~0.65us HWDGE issue).
KGROUPS = ((0, 1), (1, 2), (2, 4), (4, 8), (8, 16))
WJ_GROUPS = ((0, 16),)   # j>=1 weight prefetch granularity


def _build(nc):
    xh2 = nc.dram_tensor("xh2", [128, KT * BS], FP8, kind="ExternalInput")
    wp2 = nc.dram_tensor("wp2", [128, JT * KT * 512], FP8, kind="ExternalInput")
    bp = nc.dram_tensor("bp", [128, JT * 4], FP, kind="ExternalInput")
    cp2 = nc.dram_tensor("cp2", [128, JT * BS], FP16, kind="ExternalInput")
    hT2 = nc.dram_tensor("hT2", [128, JT * BS], FP16, kind="ExternalOutput")
    cT2 = nc.dram_tensor("cT2", [128, JT * BS], FP16, kind="ExternalOutput")

    with tile.TileContext(nc) as tc:
        with (
            tc.tile_pool(name="xh", bufs=1) as xh_pool,
            tc.tile_pool(name="w", bufs=1) as w_pool,
            tc.tile_pool(name="cp", bufs=1) as cp_pool,
            tc.tile_pool(name="gates", bufs=2) as gate_pool,
            tc.tile_pool(name="ew", bufs=2) as ew_pool,
            tc.tile_pool(name="small", bufs=1) as small_pool,
            tc.tile_pool(name="psum", bufs=1, space="PSUM") as psum_pool,
        ):
            # --- tiny setup: junk tile for PE warm-up + one bias DMA ------
            junk = small_pool.tile([128, 512], FP16, tag="junk", name="junk")
            nc.gpsimd.memset(junk[:], 0.25)
            bias_t = small_pool.tile([128, JT * 4], FP, tag="bias", name="bias")
            nc.gpsimd.dma_start(out=bias_t[:], in_=bp[:, :])

            # --- input tiles: column-sliced views of grouped DMAs ---------
            xh_g = {}     # group idx -> tile
            w_g = {}      # (j, group idx) -> tile

            def xh_view(k, n, q0=0, w=512):
                for gi, (a, b) in enumerate(KGROUPS):
                    if a <= k < b:
                        c0 = (k - a) * BS + n * 512 + q0
                        return xh_g[gi][:, c0:c0 + w]

            def w_view(j, k, g):
                if j == 0:
                    for gi, (a, b) in enumerate(KGROUPS):
                        if a <= k < b:
                            c0 = (k - a) * 512 + g * 128
                            return w_g[(0, gi)][:, c0:c0 + 128]
                for gi, (a, b) in enumerate(WJ_GROUPS):
                    if a <= k < b:
                        c0 = (k - a) * 512 + g * 128
                        return w_g[(j, gi)][:, c0:c0 + 128]

            def xh_dma(eng, gi):
                a, b = KGROUPS[gi]
                t = xh_pool.tile([128, (b - a) * BS], FP8, tag=f"xh{gi}",
                                 name=f"xh{gi}")
                eng.dma_start(out=t[:], in_=xh2[:, a * BS:b * BS])
                xh_g[gi] = t

            def w0_dma(eng, gi):
                a, b = KGROUPS[gi]
                t = w_pool.tile([128, (b - a) * 512], FP8, tag=f"w0{gi}",
                                name=f"w0{gi}")
                eng.dma_start(out=t[:], in_=wp2[:, a * 512:b * 512])
                w_g[(0, gi)] = t

            def wj_dma(j, gi):
                a, b = WJ_GROUPS[gi]
                t = w_pool.tile([128, (b - a) * 512], FP8, tag=f"w{j}{gi}",
                                name=f"w{j}{gi}")
                nc.sync.dma_start(
                    out=t[:], in_=wp2[:, (j * KT + a) * 512:(j * KT + b) * 512])
                w_g[(j, gi)] = t

            # single queue, consumption order; k0's xh lands as two halves
            # so only w k0 + the n0 half gate the first (n-major) matmuls
            w0_dma(nc.sync, 0)
            t = xh_pool.tile([128, BS], FP8, tag="xh0", name="xh0")
            nc.sync.dma_start(out=t[:, 0:512], in_=xh2[:, 0:512])
            nc.sync.dma_start(out=t[:, 512:1024], in_=xh2[:, 512:1024])
            xh_g[0] = t
            for gi in range(1, len(KGROUPS)):
                w0_dma(nc.sync, gi)
                xh_dma(nc.sync, gi)

            # --- PSUM tiles: one bank per (gate, batch-half) --------------
            def ps_tile(g, n):
                return psum_pool.tile(
                    [128, 512], FP, tag=f"ps{g}{n}", name=f"ps{g}{n}"
                )

            # --- PE warm-up on junk data while DMAs stream ----------------
            pj = ps_tile(2, 1)
            for _ in range(N_JUNK):
                nc.tensor.matmul(
                    pj[:], junk[:, 0:128], junk[:], start=True, stop=True
                )

            gates_sb = {}   # (g, n) -> fp16 SBUF tile of the current j
            tnh_sb = {}     # n -> tanh(c) tile of the current j

            def mm(ps, j, g, k, n, start, stop):
                nc.tensor.matmul(
                    ps, w_view(j, k, g), xh_view(k, n), start=start, stop=stop
                )

            def evict(j, g, ps_of_n, n_list=(0, 1)):
                func = SIG if g < 3 else TANH
                bsl = bias_t[:, j * 4 + g:j * 4 + g + 1]
                for n in n_list:
                    gt = gate_pool.tile(
                        [128, 512], FP16, tag=f"g{g}{n}", name=f"g{g}{n}_{j}"
                    )
                    nc.scalar.activation(
                        gt[:], ps_of_n[n][:], func, bias=bsl, scale=DESCALE)
                    gates_sb[(g, n)] = gt

            def chain_c(j):
                # c = fg*c_prev + ig*cc; tanh(c); DMA c out.  Runs on
                # DVE/ACT under og's matmul pass.
                for n in range(2):
                    c0 = j * BS + n * 512
                    t1 = ew_pool.tile([128, 512], FP16, tag=f"t1{n}", name=f"t1_{j}{n}")
                    nc.vector.tensor_mul(t1[:], gates_sb[(0, n)][:], gates_sb[(3, n)][:])
                    ct = ew_pool.tile([128, 512], FP16, tag=f"ct{n}", name=f"ct_{j}{n}")
                    nc.vector.tensor_mul(ct[:], gates_sb[(1, n)][:], cp_t[:, c0:c0 + 512])
                    nc.vector.tensor_add(ct[:], ct[:], t1[:])
                    tnh = ew_pool.tile([128, 512], FP16, tag=f"tnh{n}", name=f"tnh_{j}{n}")
                    nc.scalar.activation(tnh[:], ct[:], TANH)
                    tnh_sb[n] = tnh
                    nc.sync.dma_start(out=cT2[:, c0:c0 + 512], in_=ct[:])

            def chain_h(j, n_list=(0, 1)):
                for n in n_list:
                    c0 = j * BS + n * 512
                    ht = ew_pool.tile([128, 512], FP16, tag=f"ht{n}", name=f"ht_{j}{n}")
                    nc.vector.tensor_mul(ht[:], gates_sb[(2, n)][:], tnh_sb[n][:])
                    nc.sync.dma_start(out=hT2[:, c0:c0 + 512], in_=ht[:])

            # ============ j = 0: k-major (DMA-paced startup) ==============
            ps0 = {(g, n): ps_tile(g, n) for g in GORDER for n in range(2)}
            for k in range(KT):
                prio = tc.high_priority() if k == 0 else contextlib.nullcontext()
                with prio:
                    if k == 0:
                        # n-major: only the xh n0 half gates the start
                        for n in range(2):
                            for g in GORDER:
                                mm(ps0[(g, n)][:], 0, g, k, n, True, False)
                    else:
                        for g in GORDER:
                            for n in range(2):
                                mm(ps0[(g, n)][:], 0, g, k, n, False, k == KT - 1)

            # prefetch j1 weights + cp on sync (lands before j0 ends)
            wj_dma(1, 0)
            cp_t = cp_pool.tile([128, JT * BS], FP16, tag="cp", name="cp")
            nc.sync.dma_start(out=cp_t[:], in_=cp2[:, :])

            for g in GORDER:
                evict(0, g, {0: ps0[(g, 0)], 1: ps0[(g, 1)]})
            chain_c(0)
            chain_h(0)

            # ============ j >= 1: gate-major ==============================
            for j in range(1, JT):
                if j + 1 < JT:
                    for gi in range(len(WJ_GROUPS)):
                        wj_dma(j + 1, gi)
                for g in GORDER:
                    last_g = g == GORDER[-1]
                    if last_g:
                        chain_c(j)
                    if last_g and j == JT - 1:
                        # final og pass: batch-half split; last half's
                        # eviction in quarters so the post-matmul tail is
                        # minimal, with the last h DMA on the idle scalar
                        # queue
                        for n in range(2):
                            ps = ps_tile(g, n)
                            for k in range(KT):
                                mm(ps[:], j, g, k, n, k == 0, k == KT - 1)
                            if n == 0:
                                evict(j, g, {0: ps}, n_list=(0,))
                                chain_h(j, n_list=(0,))
                            else:
                                bsl = bias_t[:, j * 4 + g:j * 4 + g + 1]
                                for q in range(2):
                                    qs = slice(q * 256, (q + 1) * 256)
                                    gt = gate_pool.tile(
                                        [128, 256], FP16, tag=f"gq{q}",
                                        name=f"gq{q}")
                                    nc.scalar.activation(
                                        gt[:], ps[:, qs], SIG, bias=bsl,
                                        scale=DESCALE)
                                    ht = ew_pool.tile(
                                        [128, 256], FP16, tag=f"htq{q}",
                                        name=f"htq{q}")
                                    nc.vector.tensor_mul(
                                        ht[:], gt[:], tnh_sb[1][:, qs])
                                    c0 = j * BS + 512 + q * 256
                                    eng = nc.sync if q == 0 else nc.scalar
                                    eng.dma_start(
                                        out=hT2[:, c0:c0 + 256], in_=ht[:])
                    else:
                        psp = {0: ps_tile(g, 0), 1: ps_tile(g, 1)}
                        for k in range(KT):
                            for n in range(2):
                                mm(psp[n][:], j, g, k, n, k == 0, k == KT - 1)
                        evict(j, g, psp)
                if j < JT - 1:
                    chain_h(j)
    return nc


_NC_CACHE = None
_last_in_maps = None


def _get_nc():
    global _NC_CACHE
    if _NC_CACHE is None:
        nc = bacc.Bacc(
            "TRN2", target_bir_lowering=False, debug=False, num_devices=N_CORES
        )
        _build(nc)
        nc.compile()
        _NC_CACHE = nc
    return _NC_CACHE


# Column permutation applied to the fused [*, 4H] gate matrices, per
# hidden-column group c2: j-major, gate-minor, so each core-local 512-wide
# block j is [i_j | f_j | o_j | c_j].
def _col_index(c2):
    idx = np.empty(4 * HSH, np.int64)
    p = 0
    for j in range(JT):
        for g in range(4):
            base = g * H + c2 * HSH + j * 128
            idx[p:p + 128] = np.arange(base, base + 128)
            p += 128
    return idx


def _run_spmd_resilient(nc, in_maps):
    """Run, resetting the device once if a prior process left it wedged."""
    try:
        return run_bass_kernel_spmd(nc, in_maps, list(range(N_CORES))).results
    except Exception:
        import ctypes

        try:
            import jax

            jax.devices()
            lib = ctypes.CDLL("/opt/axon/libaxon_pjrt.so")
            lib.axon_reset.restype = ctypes.c_int64
            lib.axon_reset()
        except Exception:
            pass
        return run_bass_kernel_spmd(nc, in_maps, list(range(N_CORES))).results


def kernel(x, h_prev, c_prev, igx, igu, ib, fgx, fgu, fb, ogx, ogu, ob, cgx, cgu, cb):
    x = np.asarray(x, np.float32)
    h_prev = np.asarray(h_prev, np.float32)
    c_prev = np.asarray(c_prev, np.float32)
    igx, igu, ib = (np.asarray(a, np.float32) for a in (igx, igu, ib))
    fgx, fgu, fb = (np.asarray(a, np.float32) for a in (fgx, fgu, fb))
    ogx, ogu, ob = (np.asarray(a, np.float32) for a in (ogx, ogu, ob))
    cgx, cgu, cb = (np.asarray(a, np.float32) for a in (cgx, cgu, cb))
    nc = _get_nc()

    w_full = np.vstack([
        np.concatenate([igx, fgx, ogx, cgx], axis=1),
        np.concatenate([igu, fgu, ogu, cgu], axis=1),
    ]).astype(np.float32, copy=False)              # [2048, 4096]
    b_full = np.concatenate([ib, fb, ob, cb]).astype(np.float32, copy=False)

    wps, bps = [], []
    for c2 in range(C):
        idx = _col_index(c2)
        wperm = np.clip(w_full[:, idx] * SW, -15.0, 15.0).astype(
            ml_dtypes.float8_e3m4)                         # [2048, 2048]
        # -> [128, JT*KT*512]: w2[p, ((j*KT+k)*512)+c] = wperm[k*128+p, j*512+c]
        w2 = np.ascontiguousarray(
            wperm.reshape(KT, 128, JT, 512).transpose(1, 2, 0, 3)
            .reshape(128, JT * KT * 512))
        wps.append(w2)
        bps.append(np.ascontiguousarray(
            b_full[idx].reshape(JT * 4, 128).T.astype(np.float32)))

    in_maps = []
    for r in range(R):
        rs = slice(r * BS, (r + 1) * BS)
        xh_t = np.clip(
            np.concatenate([x[rs], h_prev[rs]], axis=1).T * SX,
            -15.0, 15.0).astype(ml_dtypes.float8_e3m4)
        # -> [128, KT*BS]: xh2[p, k*BS+c] = xh_t[k*128+p, c]
        xh2 = np.ascontiguousarray(
            xh_t.reshape(KT, 128, BS).transpose(1, 0, 2).reshape(128, KT * BS))
        for c2 in range(C):
            cp_t = c_prev[rs, c2 * HSH:(c2 + 1) * HSH].T.astype(np.float16)
            cp2 = np.ascontiguousarray(
                cp_t.reshape(JT, 128, BS).transpose(1, 0, 2).reshape(128, JT * BS))
            in_maps.append({"xh2": xh2, "wp2": wps[c2], "bp": bps[c2], "cp2": cp2})

    global _last_in_maps
    _last_in_maps = in_maps
    res = _run_spmd_resilient(nc, in_maps)

    h = np.empty((B, H), np.float32)
    c = np.empty((B, H), np.float32)
    for r in range(R):
        rs = slice(r * BS, (r + 1) * BS)
        for c2 in range(C):
            cid = r * C + c2
            cs = slice(c2 * HSH, (c2 + 1) * HSH)
            # [128, JT*BS] -> [HSH, BS] -> transpose
            hb = res[cid]["hT2"].reshape(128, JT, BS).transpose(1, 0, 2).reshape(HSH, BS)
            cbk = res[cid]["cT2"].reshape(128, JT, BS).transpose(1, 0, 2).reshape(HSH, BS)
            h[rs, cs] = hb.T.astype(np.float32)
            c[rs, cs] = cbk.T.astype(np.float32)
    return h, c


# revision 30
# speedup vs baseline: 1.0177x; 1.0177x over previous
"""Trainium2 Bass kernel for nn_FineGrainedOpLstmCellV1 (LSTM cell).

B=4096, input=1024, hidden=1024, fp32.

Strategy (v3):
- Host fuses the 8 gate matmuls into one GEMM: gates = [x|h] @ [[Wx],[Wh]].
  Shard across 8 cores as 4 batch-groups x 2 hidden-column-groups.
- Per core the GEMM is transposed (G^T = W^T @ Xh^T) so per-gate bias +
  sigmoid/tanh fuse into the PSUM->SBUF eviction (scalar.activation with
  per-partition bias); the LSTM elementwise tail runs on-chip in fp16.
- All operands are host-rearranged to [128, N] so k-tiles are column
  slices; inputs stream as a few large grouped DMAs sized to land just
  before their first consumer (HWDGE issue cost ~0.65us each, so few+big
  beats many+small at startup).
- j=0 runs k-major (DMA-paced startup); j>=1 run gate-major (cc,ig,fg,og)
  with per-(gate,half) single-bank PSUM tiles so evictions overlap the
  next gate's matmuls and j-boundaries never stall on PSUM.
- The c-chain (c = fg*cp + ig*cc, tanh(c)) is emitted before og's matmul
  pass so it hides under og; the last j's og pass is split by batch half
  and its final eviction by quarters, so only ~1.5us remains after the
  final matmul.
- Warm-up junk matmuls run during the initial DMA ramp to lift the PE
  clock gate (HAM) before real matmuls start.
- fp16 weights/acts/gates/outputs (rel err ~5e-4), fp32 PSUM accumulate.
"""

import contextlib

import ml_dtypes
import numpy as np

import concourse.bacc as bacc
import concourse.mybir as mybir
import concourse.tile as tile
from concourse.bass_utils import run_bass_kernel_spmd

FP = mybir.dt.float32
FP16 = mybir.dt.float16
FP8 = mybir.dt.float8e3
SIG = mybir.ActivationFunctionType.Sigmoid
TANH = mybir.ActivationFunctionType.Tanh

B = 4096
IN = 1024
H = 1024
R = 4              # batch groups
C = 2              # hidden-column groups
N_CORES = R * C
BS = B // R        # 1024 batch rows per core
HSH = H // C       # 512 hidden cols per core
K = IN + H         # 2048 contraction
KT = K // 128      # 16 k-tiles
JT = HSH // 128    # 4 hidden 128-row groups per core
NN = BS // 512     # 2 batch halves

GORDER = (3, 0, 1, 2)   # cc, ig, fg, og  (og last: shortest exposed tail)
N_JUNK = 4              # warm-up matmuls during the DMA ramp

# e3m4 operand scales (descaled for free in the eviction activations)
SX = 2.0                # xh pre-scale
SW = 256.0              # weight pre-scale
DESCALE = 1.0 / (SX * SW)

# k-tile groupings for the j0 input stream (each group = one DMA).  All on
# one queue in consumption order: fine at the head (group completion gates
# the first consumer), coarse later (amortize the ~0.65us HWDGE issue).
KGROUPS = ((0, 1), (1, 2), (2, 3), (3, 4), (4, 6), (6, 8), (8, 12), (12, 16))
WJ_GROUPS = ((0, 8), (8, 16))   # j>=1 weight prefetch granularity


def _build(nc):
    xh2 = nc.dram_tensor("xh2", [128, KT * BS], FP8, kind="ExternalInput")
    wp2 = nc.dram_tensor("wp2", [128, JT * KT * 512], FP8, kind="ExternalInput")
    bp = nc.dram_tensor("bp", [128, JT * 4], FP, kind="ExternalInput")
    cp2 = nc.dram_tensor("cp2", [128, JT * BS], FP16, kind="ExternalInput")
    hT2 = nc.dram_tensor("hT2", [128, JT * BS], FP16, kind="ExternalOutput")
    cT2 = nc.dram_tensor("cT2", [128, JT * BS], FP16, kind="ExternalOutput")

    with tile.TileContext(nc) as tc:
        with (
            tc.tile_pool(name="xh", bufs=1) as xh_pool,
            tc.tile_pool(name="w", bufs=1) as w_pool,
            tc.tile_pool(name="cp", bufs=1) as cp_pool,
            tc.tile_pool(name="gates", bufs=2) as gate_pool,
            tc.tile_pool(name="ew", bufs=2) as ew_pool,
            tc.tile_pool(name="small", bufs=1) as small_pool,
            tc.tile_pool(name="psum", bufs=1, space="PSUM") as psum_pool,
        ):
            # --- tiny setup: junk tile for PE warm-up + one bias DMA ------
            junk = small_pool.tile([128, 512], FP16, tag="junk", name="junk")
            nc.gpsimd.memset(junk[:], 0.25)
            bias_t = small_pool.tile([128, JT * 4], FP, tag="bias", name="bias")
            nc.gpsimd.dma_start(out=bias_t[:], in_=bp[:, :])

            # --- input tiles: column-sliced views of grouped DMAs ---------
            xh_g = {}     # group idx -> tile
            w_g = {}      # (j, group idx) -> tile

            def xh_view(k, n, q0=0, w=512):
                for gi, (a, b) in enumerate(KGROUPS):
                    if a <= k < b:
                        c0 = (k - a) * BS + n * 512 + q0
                        return xh_g[gi][:, c0:c0 + w]

            def w_view(j, k, g):
                if j == 0:
                    for gi, (a, b) in enumerate(KGROUPS):
                        if a <= k < b:
                            c0 = (k - a) * 512 + g * 128
                            return w_g[(0, gi)][:, c0:c0 + 128]
                for gi, (a, b) in enumerate(WJ_GROUPS):
                    if a <= k < b:
                        c0 = (k - a) * 512 + g * 128
                        return w_g[(j, gi)][:, c0:c0 + 128]

            def xh_dma(eng, gi):
                a, b = KGROUPS[gi]
                t = xh_pool.tile([128, (b - a) * BS], FP8, tag=f"xh{gi}",
                                 name=f"xh{gi}")
                eng.dma_start(out=t[:], in_=xh2[:, a * BS:b * BS])
                xh_g[gi] = t

            def w0_dma(eng, gi):
                a, b = KGROUPS[gi]
                t = w_pool.tile([128, (b - a) * 512], FP8, tag=f"w0{gi}",
                                name=f"w0{gi}")
                eng.dma_start(out=t[:], in_=wp2[:, a * 512:b * 512])
                w_g[(0, gi)] = t

            def wj_dma(j, gi):
                a, b = WJ_GROUPS[gi]
                t = w_pool.tile([128, (b - a) * 512], FP8, tag=f"w{j}{gi}",
                                name=f"w{j}{gi}")
                nc.sync.dma_start(
                    out=t[:], in_=wp2[:, (j * KT + a) * 512:(j * KT + b) * 512])
                w_g[(j, gi)] = t

            # single queue, consumption order; k0's xh lands as two halves
            # so only w k0 + the n0 half gate the first (n-major) matmuls
            w0_dma(nc.sync, 0)
            t = xh_pool.tile([128, BS], FP8, tag="xh0", name="xh0")
            nc.sync.dma_start(out=t[:, 0:512], in_=xh2[:, 0:512])
            nc.sync.dma_start(out=t[:, 512:1024], in_=xh2[:, 512:1024])
            xh_g[0] = t
            for gi in range(1, len(KGROUPS)):
                w0_dma(nc.sync, gi)
                xh_dma(nc.sync, gi)

            # --- PSUM tiles: one bank per (gate, batch-half) --------------
            def ps_tile(g, n):
                return psum_pool.tile(
                    [128, 512], FP, tag=f"ps{g}{n}", name=f"ps{g}{n}"
                )

            # --- PE warm-up on junk data while DMAs stream ----------------
            pj = ps_tile(2, 1)
            for _ in range(N_JUNK):
                nc.tensor.matmul(
                    pj[:], junk[:, 0:128], junk[:], start=True, stop=True
                )

            gates_sb = {}   # (g, n) -> fp16 SBUF tile of the current j
            tnh_sb = {}     # n -> tanh(c) tile of the current j

            def mm(ps, j, g, k, n, start, stop):
                nc.tensor.matmul(
                    ps, w_view(j, k, g), xh_view(k, n), start=start, stop=stop
                )

            def evict(j, g, ps_of_n, n_list=(0, 1)):
                func = SIG if g < 3 else TANH
                bsl = bias_t[:, j * 4 + g:j * 4 + g + 1]
                for n in n_list:
                    gt = gate_pool.tile(
                        [128, 512], FP16, tag=f"g{g}{n}", name=f"g{g}{n}_{j}"
                    )
                    nc.scalar.activation(
                        gt[:], ps_of_n[n][:], func, bias=bsl, scale=DESCALE)
                    gates_sb[(g, n)] = gt

            def chain_c(j):
                # c = fg*c_prev + ig*cc; tanh(c); DMA c out.  Runs on
                # DVE/ACT under og's matmul pass.
                for n in range(2):
                    c0 = j * BS + n * 512
                    t1 = ew_pool.tile([128, 512], FP16, tag=f"t1{n}", name=f"t1_{j}{n}")
                    nc.vector.tensor_mul(t1[:], gates_sb[(0, n)][:], gates_sb[(3, n)][:])
                    ct = ew_pool.tile([128, 512], FP16, tag=f"ct{n}", name=f"ct_{j}{n}")
                    nc.vector.tensor_mul(ct[:], gates_sb[(1, n)][:], cp_t[:, c0:c0 + 512])
                    nc.vector.tensor_add(ct[:], ct[:], t1[:])
                    tnh = ew_pool.tile([128, 512], FP16, tag=f"tnh{n}", name=f"tnh_{j}{n}")
                    nc.scalar.activation(tnh[:], ct[:], TANH)
                    tnh_sb[n] = tnh
                    nc.sync.dma_start(out=cT2[:, c0:c0 + 512], in_=ct[:])

            def chain_h(j, n_list=(0, 1)):
                for n in n_list:
                    c0 = j * BS + n * 512
                    ht = ew_pool.tile([128, 512], FP16, tag=f"ht{n}", name=f"ht_{j}{n}")
                    nc.vector.tensor_mul(ht[:], gates_sb[(2, n)][:], tnh_sb[n][:])
                    nc.sync.dma_start(out=hT2[:, c0:c0 + 512], in_=ht[:])

            # ============ j = 0: k-major (DMA-paced startup) ==============
            ps0 = {(g, n): ps_tile(g, n) for g in GORDER for n in range(2)}
            for k in range(KT):
                prio = tc.high_priority() if k == 0 else contextlib.nullcontext()
                with prio:
                    if k == 0:
                        # n-major: only the xh n0 half gates the start
                        for n in range(2):
                            for g in GORDER:
                                mm(ps0[(g, n)][:], 0, g, k, n, True, False)
                    else:
                        for g in GORDER:
                            for n in range(2):
                                mm(ps0[(g, n)][:], 0, g, k, n, False, k == KT - 1)

            # prefetch j1 weights + cp on sync (lands before j0 ends)
            wj_dma(1, 0)
            cp_t = cp_pool.tile([128, JT * BS], FP16, tag="cp", name="cp")
            nc.sync.dma_start(out=cp_t[:], in_=cp2[:, :])
            wj_dma(1, 1)

            for g in GORDER:
                evict(0, g, {0: ps0[(g, 0)], 1: ps0[(g, 1)]})
            chain_c(0)
            chain_h(0)

            # ============ j >= 1: gate-major ==============================
            for j in range(1, JT):
                if j + 1 < JT:
                    for gi in range(len(WJ_GROUPS)):
                        wj_dma(j + 1, gi)
                for g in GORDER:
                    last_g = g == GORDER[-1]
                    if last_g:
                        chain_c(j)
                    if last_g and j == JT - 1:
                        # final og pass: batch-half split; last half's
                        # eviction in quarters so the post-matmul tail is
                        # minimal, with the last h DMA on the idle scalar
                        # queue
                        for n in range(2):
                            ps = ps_tile(g, n)
                            for k in range(KT):
                                mm(ps[:], j, g, k, n, k == 0, k == KT - 1)
                            if n == 0:
                                evict(j, g, {0: ps}, n_list=(0,))
                                chain_h(j, n_list=(0,))
                            else:
                                bsl = bias_t[:, j * 4 + g:j * 4 + g + 1]
                                for q in range(2):
                                    qs = slice(q * 256, (q + 1) * 256)
                                    gt = gate_pool.tile(
                                        [128, 256], FP16, tag=f"gq{q}",
                                        name=f"gq{q}")
                                    nc.scalar.activation(
                                        gt[:], ps[:, qs], SIG, bias=bsl,
                                        scale=DESCALE)
                                    ht = ew_pool.tile(
                                        [128, 256], FP16, tag=f"htq{q}",
                                        name=f"htq{q}")
                                    nc.vector.tensor_mul(
                                        ht[:], gt[:], tnh_sb[1][:, qs])
                                    c0 = j * BS + 512 + q * 256
                                    eng = nc.sync if q == 0 else nc.scalar
                                    eng.dma_start(
                                        out=hT2[:, c0:c0 + 256], in_=ht[:])
                    else:
                        psp = {0: ps_tile(g, 0), 1: ps_tile(g, 1)}
                        for k in range(KT):
                            for n in range(2):
                                mm(psp[n][:], j, g, k, n, k == 0, k == KT - 1)
                        evict(j, g, psp)
                if j < JT - 1:
                    chain_h(j)
    return nc


_NC_CACHE = None
_last_in_maps = None


def _get_nc():
    global _NC_CACHE
    if _NC_CACHE is None:
        nc = bacc.Bacc(
            "TRN2", target_bir_lowering=False, debug=False, num_devices=N_CORES
        )
        _build(nc)
        nc.compile()
        _NC_CACHE = nc
    return _NC_CACHE


# Column permutation applied to the fused [*, 4H] gate matrices, per
# hidden-column group c2: j-major, gate-minor, so each core-local 512-wide
# block j is [i_j | f_j | o_j | c_j].
def _col_index(c2):
    idx = np.empty(4 * HSH, np.int64)
    p = 0
    for j in range(JT):
        for g in range(4):
            base = g * H + c2 * HSH + j * 128
            idx[p:p + 128] = np.arange(base, base + 128)
            p += 128
    return idx


def _run_spmd_resilient(nc, in_maps):
    """Run, resetting the device once if a prior process left it wedged."""
    try:
        return run_bass_kernel_spmd(nc, in_maps, list(range(N_CORES))).results
    except Exception:
        import ctypes

        try:
            import jax

            jax.devices()
            lib = ctypes.CDLL("/opt/axon/libaxon_pjrt.so")
            lib.axon_reset.restype = ctypes.c_int64
            lib.axon_reset()
        except Exception:
            pass
        return run_bass_kernel_spmd(nc, in_maps, list(range(N_CORES))).results


def kernel(x, h_prev, c_prev, igx, igu, ib, fgx, fgu, fb, ogx, ogu, ob, cgx, cgu, cb):
    x = np.asarray(x, np.float32)
    h_prev = np.asarray(h_prev, np.float32)
    c_prev = np.asarray(c_prev, np.float32)
    igx, igu, ib = (np.asarray(a, np.float32) for a in (igx, igu, ib))
    fgx, fgu, fb = (np.asarray(a, np.float32) for a in (fgx, fgu, fb))
    ogx, ogu, ob = (np.asarray(a, np.float32) for a in (ogx, ogu, ob))
    cgx, cgu, cb = (np.asarray(a, np.float32) for a in (cgx, cgu, cb))
    nc = _get_nc()

    w_full = np.vstack([
        np.concatenate([igx, fgx, ogx, cgx], axis=1),
        np.concatenate([igu, fgu, ogu, cgu], axis=1),
    ]).astype(np.float32, copy=False)              # [2048, 4096]
    b_full = np.concatenate([ib, fb, ob, cb]).astype(np.float32, copy=False)

    wps, bps = [], []
    for c2 in range(C):
        idx = _col_index(c2)
        wperm = np.clip(w_full[:, idx] * SW, -15.0, 15.0).astype(
            ml_dtypes.float8_e3m4)                         # [2048, 2048]
        # -> [128, JT*KT*512]: w2[p, ((j*KT+k)*512)+c] = wperm[k*128+p, j*512+c]
        w2 = np.ascontiguousarray(
            wperm.reshape(KT, 128, JT, 512).transpose(1, 2, 0, 3)
            .reshape(128, JT * KT * 512))
        wps.append(w2)
        bps.append(np.ascontiguousarray(
            b_full[idx].reshape(JT * 4, 128).T.astype(np.float32)))

    in_maps = []
    for r in range(R):
        rs = slice(r * BS, (r + 1) * BS)
        xh_t = np.clip(
            np.concatenate([x[rs], h_prev[rs]], axis=1).T * SX,
            -15.0, 15.0).astype(ml_dtypes.float8_e3m4)
        # -> [128, KT*BS]: xh2[p, k*BS+c] = xh_t[k*128+p, c]
        xh2 = np.ascontiguousarray(
            xh_t.reshape(KT, 128, BS).transpose(1, 0, 2).reshape(128, KT * BS))
        for c2 in range(C):
            cp_t = c_prev[rs, c2 * HSH:(c2 + 1) * HSH].T.astype(np.float16)
            cp2 = np.ascontiguousarray(
                cp_t.reshape(JT, 128, BS).transpose(1, 0, 2).reshape(128, JT * BS))
            in_maps.append({"xh2": xh2, "wp2": wps[c2], "bp": bps[c2], "cp2": cp2})

    global _last_in_maps
    _last_in_maps = in_maps
    res = _run_spmd_resilient(nc, in_maps)

    h = np.empty((B, H), np.float32)
    c = np.empty((B, H), np.float32)
    for r in range(R):
        rs = slice(r * BS, (r + 1) * BS)
        for c2 in range(C):
            cid = r * C + c2
            cs = slice(c2 * HSH, (c2 + 1) * HSH)
            # [128, JT*BS] -> [HSH, BS] -> transpose
            hb = res[cid]["hT2"].reshape(128, JT, BS).transpose(1, 0, 2).reshape(HSH, BS)
            cbk = res[cid]["cT2"].reshape(128, JT, BS).transpose(1, 0, 2).reshape(HSH, BS)
            h[rs, cs] = hb.T.astype(np.float32)
            c[rs, cs] = cbk.T.astype(np.float32)
    return h, c


# revision 33
# speedup vs baseline: 1.0182x; 1.0004x over previous
"""Trainium2 Bass kernel for nn_FineGrainedOpLstmCellV1 (LSTM cell).

B=4096, input=1024, hidden=1024, fp32.

Strategy (v4):
- Host fuses the 8 gate matmuls into one GEMM: gates = [x|h] @ [[Wx],[Wh]].
  Shard across 8 cores as 4 batch-groups x 2 hidden-column-groups.
- Per core the GEMM is transposed (G^T = W^T @ Xh^T) so per-gate bias +
  sigmoid/tanh fuse into the PSUM->SBUF eviction (scalar.activation with
  per-partition bias); the LSTM elementwise tail runs on-chip in fp16.
- All operands are host-rearranged to [128, N] so k-tiles are column
  slices; inputs stream as a few large grouped DMAs sized to land just
  before their first consumer (HWDGE issue cost ~0.65us each, so few+big
  beats many+small at startup).
- j=0 runs k-major (DMA-paced startup); j>=1 run gate-major (cc,ig,fg,og)
  with per-(gate,half) single-bank PSUM tiles so evictions overlap the
  next gate's matmuls and j-boundaries never stall on PSUM.
- The c-chain (c = fg*cp + ig*cc, tanh(c)) is emitted before og's matmul
  pass so it hides under og; the last j's og pass is split by batch half
  and its final eviction by quarters, so only ~1.5us remains after the
  final matmul.
- Warm-up junk matmuls run during the initial DMA ramp to lift the PE
  clock gate (HAM) before real matmuls start.
- GEMM operands in fp8 e3m4 (same 1 cycle/row as fp16 on the PE, but half
  the DMA bytes -> faster startup and lighter HBM): host pre-scales
  xh by 2 and W by 256, and the per-gate eviction activation descales by
  1/512 for free via its scale parameter.  Gates/c_prev/outputs fp16,
  fp32 PSUM accumulate.  Deterministic rel err 1.29e-2 (gate 2e-2).
"""

import contextlib

import ml_dtypes
import numpy as np

import concourse.bacc as bacc
import concourse.mybir as mybir
import concourse.tile as tile
from concourse.bass_utils import run_bass_kernel_spmd

FP = mybir.dt.float32
FP16 = mybir.dt.float16
FP8 = mybir.dt.float8e3
SIG = mybir.ActivationFunctionType.Sigmoid
TANH = mybir.ActivationFunctionType.Tanh

B = 4096
IN = 1024
H = 1024
R = 4              # batch groups
C = 2              # hidden-column groups
N_CORES = R * C
BS = B // R        # 1024 batch rows per core
HSH = H // C       # 512 hidden cols per core
K = IN + H         # 2048 contraction
KT = K // 128      # 16 k-tiles
JT = HSH // 128    # 4 hidden 128-row groups per core
NN = BS // 512     # 2 batch halves

GORDER = (3, 0, 1, 2)   # cc, ig, fg, og  (og last: shortest exposed tail)
N_JUNK = 4              # warm-up matmuls during the DMA ramp

# e3m4 operand scales (descaled for free in the eviction activations)
SX = 2.0                # xh pre-scale
SW = 256.0              # weight pre-scale
DESCALE = 1.0 / (SX * SW)

# k-tile groupings for the j0 input stream (each group = one DMA).  All on
# one queue in consumption order: fine at the head (group completion gates
# the first consumer), coarse later (amortize the ~0.65us HWDGE issue).
KGROUPS = ((0, 1), (1, 2), (2, 3), (3, 4), (4, 6), (6, 8), (8, 12), (12, 16))
WJ_GROUPS = ((0, 8), (8, 16))   # j>=1 weight prefetch granularity


def _build(nc):
    xh2 = nc.dram_tensor("xh2", [128, KT * BS], FP8, kind="ExternalInput")
    wp2 = nc.dram_tensor("wp2", [128, JT * KT * 512], FP8, kind="ExternalInput")
    bp = nc.dram_tensor("bp", [128, JT * 4], FP, kind="ExternalInput")
    cp2 = nc.dram_tensor("cp2", [128, JT * BS], FP16, kind="ExternalInput")
    hT2 = nc.dram_tensor("hT2", [128, JT * BS], FP16, kind="ExternalOutput")
    cT2 = nc.dram_tensor("cT2", [128, JT * BS], FP16, kind="ExternalOutput")

    with tile.TileContext(nc) as tc:
        with (
            tc.tile_pool(name="xh", bufs=1) as xh_pool,
            tc.tile_pool(name="w", bufs=1) as w_pool,
            tc.tile_pool(name="cp", bufs=1) as cp_pool,
            tc.tile_pool(name="gates", bufs=2) as gate_pool,
            tc.tile_pool(name="ew", bufs=2) as ew_pool,
            tc.tile_pool(name="small", bufs=1) as small_pool,
            tc.tile_pool(name="psum", bufs=1, space="PSUM") as psum_pool,
        ):
            # --- tiny setup: junk tile for PE warm-up + one bias DMA ------
            junk = small_pool.tile([128, 512], FP16, tag="junk", name="junk")
            nc.gpsimd.memset(junk[:], 0.25)
            bias_t = small_pool.tile([128, JT * 4], FP, tag="bias", name="bias")
            nc.gpsimd.dma_start(out=bias_t[:], in_=bp[:, :])

            # --- input tiles: column-sliced views of grouped DMAs ---------
            xh_g = {}     # group idx -> tile
            w_g = {}      # (j, group idx) -> tile

            def xh_view(k, n, q0=0, w=512):
                for gi, (a, b) in enumerate(KGROUPS):
                    if a <= k < b:
                        c0 = (k - a) * BS + n * 512 + q0
                        return xh_g[gi][:, c0:c0 + w]

            def w_view(j, k, g):
                if j == 0:
                    for gi, (a, b) in enumerate(KGROUPS):
                        if a <= k < b:
                            c0 = (k - a) * 512 + g * 128
                            return w_g[(0, gi)][:, c0:c0 + 128]
                for gi, (a, b) in enumerate(WJ_GROUPS):
                    if a <= k < b:
                        c0 = (k - a) * 512 + g * 128
                        return w_g[(j, gi)][:, c0:c0 + 128]

            def xh_dma(eng, gi):
                a, b = KGROUPS[gi]
                t = xh_pool.tile([128, (b - a) * BS], FP8, tag=f"xh{gi}",
                                 name=f"xh{gi}")
                eng.dma_start(out=t[:], in_=xh2[:, a * BS:b * BS])
                xh_g[gi] = t

            def w0_dma(eng, gi):
                a, b = KGROUPS[gi]
                t = w_pool.tile([128, (b - a) * 512], FP8, tag=f"w0{gi}",
                                name=f"w0{gi}")
                eng.dma_start(out=t[:], in_=wp2[:, a * 512:b * 512])
                w_g[(0, gi)] = t

            def wj_dma(j, gi):
                a, b = WJ_GROUPS[gi]
                t = w_pool.tile([128, (b - a) * 512], FP8, tag=f"w{j}{gi}",
                                name=f"w{j}{gi}")
                nc.sync.dma_start(
                    out=t[:], in_=wp2[:, (j * KT + a) * 512:(j * KT + b) * 512])
                w_g[(j, gi)] = t

            # single queue, consumption order; k0's xh lands as two halves
            # so only w k0 + the n0 half gate the first (n-major) matmuls
            w0_dma(nc.sync, 0)
            t = xh_pool.tile([128, BS], FP8, tag="xh0", name="xh0")
            nc.sync.dma_start(out=t[:, 0:512], in_=xh2[:, 0:512])
            nc.sync.dma_start(out=t[:, 512:1024], in_=xh2[:, 512:1024])
            xh_g[0] = t
            for gi in range(1, len(KGROUPS)):
                w0_dma(nc.sync, gi)
                xh_dma(nc.sync, gi)

            # --- PSUM tiles: one bank per (gate, batch-half) --------------
            def ps_tile(g, n):
                return psum_pool.tile(
                    [128, 512], FP, tag=f"ps{g}{n}", name=f"ps{g}{n}"
                )

            # --- PE warm-up on junk data while DMAs stream ----------------
            pj = ps_tile(2, 1)
            for _ in range(N_JUNK):
                nc.tensor.matmul(
                    pj[:], junk[:, 0:128], junk[:], start=True, stop=True
                )

            gates_sb = {}   # (g, n) -> fp16 SBUF tile of the current j
            tnh_sb = {}     # n -> tanh(c) tile of the current j

            def mm(ps, j, g, k, n, start, stop):
                nc.tensor.matmul(
                    ps, w_view(j, k, g), xh_view(k, n), start=start, stop=stop
                )

            def evict(j, g, ps_of_n, n_list=(0, 1)):
                func = SIG if g < 3 else TANH
                bsl = bias_t[:, j * 4 + g:j * 4 + g + 1]
                for n in n_list:
                    gt = gate_pool.tile(
                        [128, 512], FP16, tag=f"g{g}{n}", name=f"g{g}{n}_{j}"
                    )
                    nc.scalar.activation(
                        gt[:], ps_of_n[n][:], func, bias=bsl, scale=DESCALE)
                    gates_sb[(g, n)] = gt

            def chain_c(j):
                # c = fg*c_prev + ig*cc; tanh(c); DMA c out.  Runs on
                # DVE/ACT under og's matmul pass.
                for n in range(2):
                    c0 = j * BS + n * 512
                    t1 = ew_pool.tile([128, 512], FP16, tag=f"t1{n}", name=f"t1_{j}{n}")
                    nc.vector.tensor_mul(t1[:], gates_sb[(0, n)][:], gates_sb[(3, n)][:])
                    ct = ew_pool.tile([128, 512], FP16, tag=f"ct{n}", name=f"ct_{j}{n}")
                    nc.vector.tensor_mul(ct[:], gates_sb[(1, n)][:], cp_t[:, c0:c0 + 512])
                    nc.vector.tensor_add(ct[:], ct[:], t1[:])
                    tnh = ew_pool.tile([128, 512], FP16, tag=f"tnh{n}", name=f"tnh_{j}{n}")
                    nc.scalar.activation(tnh[:], ct[:], TANH)
                    tnh_sb[n] = tnh
                    nc.sync.dma_start(out=cT2[:, c0:c0 + 512], in_=ct[:])

            def chain_h(j, n_list=(0, 1)):
                for n in n_list:
                    c0 = j * BS + n * 512
                    ht = ew_pool.tile([128, 512], FP16, tag=f"ht{n}", name=f"ht_{j}{n}")
                    nc.vector.tensor_mul(ht[:], gates_sb[(2, n)][:], tnh_sb[n][:])
                    nc.sync.dma_start(out=hT2[:, c0:c0 + 512], in_=ht[:])

            # ============ j = 0: k-major (DMA-paced startup) ==============
            ps0 = {(g, n): ps_tile(g, n) for g in GORDER for n in range(2)}
            for k in range(KT):
                prio = tc.high_priority() if k == 0 else contextlib.nullcontext()
                with prio:
                    if k == 0:
                        # n-major: only the xh n0 half gates the start
                        for n in range(2):
                            for g in GORDER:
                                mm(ps0[(g, n)][:], 0, g, k, n, True, False)
                    else:
                        for g in GORDER:
                            for n in range(2):
                                mm(ps0[(g, n)][:], 0, g, k, n, False, k == KT - 1)

            # prefetch j1 weights + cp on sync (lands before j0 ends)
            wj_dma(1, 0)
            cp_t = cp_pool.tile([128, JT * BS], FP16, tag="cp", name="cp")
            nc.sync.dma_start(out=cp_t[:], in_=cp2[:, :])
            wj_dma(1, 1)

            for g in GORDER:
                evict(0, g, {0: ps0[(g, 0)], 1: ps0[(g, 1)]})
            chain_c(0)
            chain_h(0)

            # ============ j >= 1: gate-major ==============================
            for j in range(1, JT):
                if j + 1 < JT:
                    for gi in range(len(WJ_GROUPS)):
                        wj_dma(j + 1, gi)
                for g in GORDER:
                    last_g = g == GORDER[-1]
                    if last_g:
                        chain_c(j)
                    if last_g and j == JT - 1:
                        # final og pass: batch-half split; last half's
                        # eviction in quarters so the post-matmul tail is
                        # minimal, with the last h DMA on the idle scalar
                        # queue
                        for n in range(2):
                            ps = ps_tile(g, n)
                            for k in range(KT):
                                mm(ps[:], j, g, k, n, k == 0, k == KT - 1)
                            if n == 0:
                                evict(j, g, {0: ps}, n_list=(0,))
                                chain_h(j, n_list=(0,))
                            else:
                                bsl = bias_t[:, j * 4 + g:j * 4 + g + 1]
                                for q in range(2):
                                    qs = slice(q * 256, (q + 1) * 256)
                                    gt = gate_pool.tile(
                                        [128, 256], FP16, tag=f"gq{q}",
                                        name=f"gq{q}")
                                    nc.scalar.activation(
                                        gt[:], ps[:, qs], SIG, bias=bsl,
                                        scale=DESCALE)
                                    ht = ew_pool.tile(
                                        [128, 256], FP16, tag=f"htq{q}",
                                        name=f"htq{q}")
                                    nc.vector.tensor_mul(
                                        ht[:], gt[:], tnh_sb[1][:, qs])
                                    c0 = j * BS + 512 + q * 256
                                    eng = nc.sync if q == 0 else nc.scalar
                                    eng.dma_start(
                                        out=hT2[:, c0:c0 + 256], in_=ht[:])
                    else:
                        psp = {0: ps_tile(g, 0), 1: ps_tile(g, 1)}
                        for k in range(KT):
                            for n in range(2):
                                mm(psp[n][:], j, g, k, n, k == 0, k == KT - 1)
                        evict(j, g, psp)
                if j < JT - 1:
                    chain_h(j)
    return nc


_NC_CACHE = None
_last_in_maps = None


def _get_nc():
    global _NC_CACHE
    if _NC_CACHE is None:
        nc = bacc.Bacc(
            "TRN2", target_bir_lowering=False, debug=False, num_devices=N_CORES
        )
        _build(nc)
        nc.compile()
        _NC_CACHE = nc
    return _NC_CACHE


# Column permutation applied to the fused [*, 4H] gate matrices, per
# hidden-column group c2: j-major, gate-minor, so each core-local 512-wide
# block j is [i_j | f_j | o_j | c_j].
def _col_index(c2):
    idx = np.empty(4 * HSH, np.int64)
    p = 0
    for j in range(JT):
        for g in range(4):
            base = g * H + c2 * HSH + j * 128
            idx[p:p + 128] = np.arange(base, base + 128)
            p += 128
    return idx


def _run_spmd_resilient(nc, in_maps):
    """Run, resetting the device once if a prior process left it wedged."""
    try:
        return run_bass_kernel_spmd(nc, in_maps, list(range(N_CORES))).results
    except Exception:
        import ctypes

        try:
            import jax

            jax.devices()
            lib = ctypes.CDLL("/opt/axon/libaxon_pjrt.so")
            lib.axon_reset.restype = ctypes.c_int64
            lib.axon_reset()
        except Exception:
            pass
        return run_bass_kernel_spmd(nc, in_maps, list(range(N_CORES))).results


def kernel(x, h_prev, c_prev, igx, igu, ib, fgx, fgu, fb, ogx, ogu, ob, cgx, cgu, cb):
    x = np.asarray(x, np.float32)
    h_prev = np.asarray(h_prev, np.float32)
    c_prev = np.asarray(c_prev, np.float32)
    igx, igu, ib = (np.asarray(a, np.float32) for a in (igx, igu, ib))
    fgx, fgu, fb = (np.asarray(a, np.float32) for a in (fgx, fgu, fb))
    ogx, ogu, ob = (np.asarray(a, np.float32) for a in (ogx, ogu, ob))
    cgx, cgu, cb = (np.asarray(a, np.float32) for a in (cgx, cgu, cb))
    nc = _get_nc()

    w_full = np.vstack([
        np.concatenate([igx, fgx, ogx, cgx], axis=1),
        np.concatenate([igu, fgu, ogu, cgu], axis=1),
    ]).astype(np.float32, copy=False)              # [2048, 4096]
    b_full = np.concatenate([ib, fb, ob, cb]).astype(np.float32, copy=False)

    wps, bps = [], []
    for c2 in range(C):
        idx = _col_index(c2)
        wperm = np.clip(w_full[:, idx] * SW, -15.0, 15.0).astype(
            ml_dtypes.float8_e3m4)                         # [2048, 2048]
        # -> [128, JT*KT*512]: w2[p, ((j*KT+k)*512)+c] = wperm[k*128+p, j*512+c]
        w2 = np.ascontiguousarray(
            wperm.reshape(KT, 128, JT, 512).transpose(1, 2, 0, 3)
            .reshape(128, JT * KT * 512))
        wps.append(w2)
        bps.append(np.ascontiguousarray(
            b_full[idx].reshape(JT * 4, 128).T.astype(np.float32)))

    in_maps = []
    for r in range(R):
        rs = slice(r * BS, (r + 1) * BS)
        xh_t = np.clip(
            np.concatenate([x[rs], h_prev[rs]], axis=1).T * SX,
            -15.0, 15.0).astype(ml_dtypes.float8_e3m4)
        # -> [128, KT*BS]: xh2[p, k*BS+c] = xh_t[k*128+p, c]
        xh2 = np.ascontiguousarray(
            xh_t.reshape(KT, 128, BS).transpose(1, 0, 2).reshape(128, KT * BS))
        for c2 in range(C):
            cp_t = c_prev[rs, c2 * HSH:(c2 + 1) * HSH].T.astype(np.float16)
            cp2 = np.ascontiguousarray(
                cp_t.reshape(JT, 128, BS).transpose(1, 0, 2).reshape(128, JT * BS))
            in_maps.append({"xh2": xh2, "wp2": wps[c2], "bp": bps[c2], "cp2": cp2})

    global _last_in_maps
    _last_in_maps = in_maps
    res = _run_spmd_resilient(nc, in_maps)

    h = np.empty((B, H), np.float32)
    c = np.empty((B, H), np.float32)
    for r in range(R):
        rs = slice(r * BS, (r + 1) * BS)
        for c2 in range(C):
            cid = r * C + c2
            cs = slice(c2 * HSH, (c2 + 1) * HSH)
            # [128, JT*BS] -> [HSH, BS] -> transpose
            hb = res[cid]["hT2"].reshape(128, JT, BS).transpose(1, 0, 2).reshape(HSH, BS)
            cbk = res[cid]["cT2"].reshape(128, JT, BS).transpose(1, 0, 2).reshape(HSH, BS)
            h[rs, cs] = hb.T.astype(np.float32)
            c[rs, cs] = cbk.T.astype(np.float32)
    return h, c
